# revision 12
# baseline (speedup 1.0000x reference)
"""CrossOnlyAttention Trainium2 kernel.

Data-parallel over batch: 64 batches -> 8 cores x 8 batches. Everything fp32r.

Per-core dataflow (per batch):
  x^T [C,T] in SBUF -> QKV projections:
     Q^T,K^T head-major [2 heads x 64, T] per head-pair (bias + 1/8 scale fused
     into the PSUM->SBUF copy), V token-major [T, 64] per head with a ones
     column appended (V_aug) so the PV matmul also produces the softmax
     denominator Z as row 64.
  Scores computed transposed S^T[k,q] = K^T.T @ Q^T (contraction=64, the two
     heads of a pair run on disjoint PE row groups). Softmax needs no
     max-subtraction (|logits| < ~4). The additive cross-mask reduces, up to a
     per-query constant that softmax cancels, to a per-KEY bias that only
     depends on which image the query is in -> folded into the exp() bias
     operand per free-dim slice ([1:235] vs [235:469]).
  PV: Y^T[65,q] = V_aug.T @ E accumulated over 4 k-tiles; row 64 = Z.
  Normalize: R=1/Z (DVE), broadcast R across 64 partitions with a K=1 matmul,
     DVE multiply -> Yhat^T [C,T] (odd heads shifted to partitions 64:128 via
     SBUF->SBUF DMA). V-bias is folded into b_proj on the host (attention rows
     sum to 1).
  Proj: out[t, c] accumulated over 8 cin tiles + a K=1 matmul adding
     b_proj_eff via a ones row.

fp32r ISA restrictions (walrus s3d3_mm_fp32r): every matmul operand free-dim
count and the PSUM dst free count must be EVEN, dst start_partition must be 0.
So token free dims are padded 469->470 (TE) and ragged stationary token slices
85->86 (STT); consumers only ever read the real 469/85 region, and the one
place padding could leak into real results (a fake key token entering the PV
reduction) is avoided because PV contracts over real partitions (85) only.
"""

import os
import sys

import numpy as np

for _p in (
    "/opt/trn_rl_repo",
    "/root/.axon_site",
    "/root/.axon_site/_ro/trn_rl_repo",
    "/root/.axon_site/_ro/pypackages",
):
    if os.path.isdir(_p) and _p not in sys.path:
        sys.path.append(_p)

import concourse.bass as bass  # noqa: E402,F401
import concourse.tile as tile  # noqa: E402
from concourse import bacc, mybir  # noqa: E402
from concourse.bass_utils import run_bass_kernel_spmd  # noqa: E402

B, T, C = 64, 469, 1024
H, HD = 16, 64
T1 = 234
NCORES = 8
BL = B // NCORES
F32R = mybir.dt.float32r
F32 = mybir.dt.float32
TT = [128, 128, 128, 85]    # real token-tile sizes
STT = [128, 128, 128, 86]   # even-padded stationary slice sizes (fp32r ISA)
TOFF = [0, 128, 256, 384]
TE = 470                    # even-padded T for matmul free dims
NKT = 8  # cin contraction tiles (1024/128)
IDENT = mybir.ActivationFunctionType.Identity
EXP = mybir.ActivationFunctionType.Exp

_cache = {}


def _build():
    nc = bacc.Bacc(trn_type="TRN2", name="xattn")
    x_h = nc.dram_tensor("x", [BL, T, C], F32R, kind="ExternalInput")
    wa_h = nc.dram_tensor("wa", [C, 3 * C], F32R, kind="ExternalInput")
    wp_h = nc.dram_tensor("wp", [C, C], F32R, kind="ExternalInput")
    bqs_h = nc.dram_tensor("bqs", [C], F32, kind="ExternalInput")
    bk_h = nc.dram_tensor("bk", [C], F32, kind="ExternalInput")
    bpe_h = nc.dram_tensor("bpe", [C], F32R, kind="ExternalInput")
    mb_h = nc.dram_tensor("mb", [128, 4, 2], F32, kind="ExternalInput")
    out_h = nc.dram_tensor("out", [BL, T, C], F32, kind="ExternalOutput")

    with tile.TileContext(nc) as tc:
        with (
            tc.tile_pool(name="singles", bufs=1) as singles,
            tc.tile_pool(name="xy", bufs=2) as xy_pool,
            tc.tile_pool(name="qk", bufs=4) as qk_pool,
            tc.tile_pool(name="ep", bufs=3) as e_pool,
            tc.tile_pool(name="vw", bufs=2) as vw_pool,
            tc.tile_pool(name="pw", bufs=2) as pw_pool,
            tc.tile_pool(name="rp", bufs=2) as r_pool,
            tc.tile_pool(name="tp", bufs=2) as t_pool,
            tc.tile_pool(name="yp", bufs=4) as y_pool,
            tc.tile_pool(name="acc", bufs=3, space="PSUM") as acc_pool,
            tc.tile_pool(name="sp", bufs=3, space="PSUM") as s_pool,
            tc.tile_pool(name="pvp", bufs=2, space="PSUM") as pv_pool,
        ):
            # ---- resident constants ----
            wqk = singles.tile([128, NKT, 2 * C], F32R)
            nc.sync.dma_start(
                wqk[:], wa_h[:, 0 : 2 * C].rearrange("(kt p) n -> p kt n", p=128)
            )
            ones32 = singles.tile([128, 128], F32)
            nc.vector.memset(ones32[:], 1.0)
            ones = singles.tile([128, 128], F32R)
            nc.scalar.copy(ones[:], ones32[:])
            zeros32 = singles.tile([128, NKT], F32)
            nc.vector.memset(zeros32[:], 0.0)
            onz = singles.tile([128, H, 2], F32)
            nc.vector.memset(onz[:, :, 0:1], 1.0)
            nc.vector.memset(onz[:, :, 1:2], 0.0)
            vsb = singles.tile([128, 4, H, 66], F32R)
            for _tt in range(4):
                nc.scalar.copy(vsb[:, _tt, :, 64:66], onz[:])
            bqs = singles.tile([128, NKT], F32)
            nc.sync.dma_start(bqs[:], bqs_h.ap().rearrange("(cb p) -> p cb", p=128))
            bk = singles.tile([128, NKT], F32)
            nc.sync.dma_start(bk[:], bk_h.ap().rearrange("(cb p) -> p cb", p=128))
            bpe = singles.tile([1, C], F32R)
            nc.sync.dma_start(bpe[:], bpe_h.ap().unsqueeze(0))
            mb = singles.tile([128, 4, 2], F32)
            nc.sync.dma_start(mb[:], mb_h[:])

            for b in range(BL):
                xT = xy_pool.tile([128, NKT, TE], F32R, tag="xy")
                for cs in range(NKT):
                    nc.sync.dma_start(
                        xT[:, cs, 0:T],
                        x_h[b][:, 128 * cs : 128 * (cs + 1)].transpose([1, 0]),
                    )
                nc.scalar.copy(xT[:, :, T:TE], zeros32[:].unsqueeze(2))
                yh = xy_pool.tile([128, NKT, TE], F32R, tag="xy")
                nc.scalar.copy(yh[:, :, T:TE], zeros32[:].unsqueeze(2))

                for c in range(4):
                    # V projection chunk: heads 4c..4c+3 (vcols 256c..256c+256)
                    vw = vw_pool.tile([128, NKT, 256], F32R, tag="vw")
                    nc.sync.dma_start(
                        vw[:],
                        wa_h[:, 2 * C + 256 * c : 2 * C + 256 * (c + 1)].rearrange(
                            "(kt p) n -> p kt n", p=128
                        ),
                    )
                    for tt in range(4):
                        n, sn = TT[tt], STT[tt]
                        acc = acc_pool.tile([128, TE], F32, tag="acc")
                        for kt in range(NKT):
                            nc.tensor.matmul(
                                acc[0:sn, 0:256],
                                lhsT=xT[:, kt, TOFF[tt] : TOFF[tt] + sn],
                                rhs=vw[:, kt, :],
                                start=(kt == 0),
                                stop=(kt == NKT - 1),
                            )
                        nc.scalar.copy(
                            vsb[0:n, tt, 4 * c : 4 * c + 4, 0:64],
                            acc[0:n, 0:256].rearrange("p (h d) -> p h d", h=4),
                        )

                    for hp in (2 * c, 2 * c + 1):
                        # Q^T / K^T for head pair hp (heads 2hp, 2hp+1)
                        qacc = acc_pool.tile([128, TE], F32, tag="acc")
                        for kt in range(NKT):
                            nc.tensor.matmul(
                                qacc[:, 0:TE],
                                lhsT=wqk[:, kt, 128 * hp : 128 * hp + 128],
                                rhs=xT[:, kt, 0:TE],
                                start=(kt == 0),
                                stop=(kt == NKT - 1),
                            )
                        qsb = qk_pool.tile([128, TE], F32R, tag="qk")
                        nc.scalar.activation(
                            qsb[:], qacc[:, 0:TE], IDENT,
                            bias=bqs[:, hp : hp + 1], scale=0.125,
                        )
                        kacc = acc_pool.tile([128, TE], F32, tag="acc")
                        for kt in range(NKT):
                            nc.tensor.matmul(
                                kacc[:, 0:TE],
                                lhsT=wqk[:, kt, C + 128 * hp : C + 128 * hp + 128],
                                rhs=xT[:, kt, 0:TE],
                                start=(kt == 0),
                                stop=(kt == NKT - 1),
                            )
                        ksb = qk_pool.tile([128, TE], F32R, tag="qk")
                        nc.scalar.activation(
                            ksb[:], kacc[:, 0:TE], IDENT,
                            bias=bk[:, hp : hp + 1], scale=1.0,
                        )

                        for sub in range(2):
                            h = 2 * hp + sub
                            r0 = 64 * sub
                            pv = pv_pool.tile([66, TE], F32, tag="pv")
                            for kt in range(4):
                                n, sn = TT[kt], STT[kt]
                                ko = TOFF[kt]
                                s = s_pool.tile([128, TE], F32, tag="s")
                                nc.tensor.matmul(
                                    s[0:sn, 0:TE],
                                    lhsT=ksb[r0 : r0 + 64, ko : ko + sn],
                                    rhs=qsb[r0 : r0 + 64, 0:TE],
                                    start=True,
                                    stop=True,
                                )
                                e = e_pool.tile([128, TE], F32R, tag="e")
                                nc.scalar.activation(e[0:n, 0:1], s[0:n, 0:1], EXP)
                                nc.scalar.activation(
                                    e[0:n, 1 : T1 + 1], s[0:n, 1 : T1 + 1], EXP,
                                    bias=mb[0:n, kt, 0:1],
                                )
                                nc.scalar.activation(
                                    e[0:n, T1 + 1 : TE], s[0:n, T1 + 1 : TE], EXP,
                                    bias=mb[0:n, kt, 1:2],
                                )
                                nc.tensor.matmul(
                                    pv[:, 0:TE],
                                    lhsT=vsb[0:n, kt, h, 0:66],
                                    rhs=e[0:n, 0:TE],
                                    start=(kt == 0),
                                    stop=(kt == 3),
                                )
                            r = r_pool.tile([65, TE], F32R, tag="r")
                            with nc.allow_low_precision(
                                reason="fp32r reciprocal feeds fp32r matmul"
                            ):
                                nc.vector.reciprocal(
                                    r[64:65, 0:TE], pv[64:65, 0:TE]
                                )
                            bc = s_pool.tile([128, TE], F32, tag="s")
                            nc.tensor.matmul(
                                bc[0:64, 0:TE],
                                lhsT=ones[64:65, 0:64],
                                rhs=r[64:65, 0:TE],
                                start=True,
                                stop=True,
                            )
                            bcs = t_pool.tile([64, T], F32, tag="bcs")
                            nc.scalar.copy(bcs[:, 0:T], bc[0:64, 0:T])
                            if sub == 0:
                                nc.vector.tensor_mul(
                                    yh[0:64, hp, 0:T], pv[0:64, 0:T], bcs[:, 0:T]
                                )
                            else:
                                tmp = t_pool.tile([64, T], F32R, tag="tmp")
                                nc.vector.tensor_mul(
                                    tmp[:, 0:T], pv[0:64, 0:T], bcs[:, 0:T]
                                )
                                nc.sync.dma_start(yh[64:128, hp, 0:T], tmp[:, 0:T])

                # ---- output projection ----
                ytiles = []
                for _tt in range(4):
                    ytile = y_pool.tile([128, C], F32, tag="y", name=f"y{b}_{_tt}")
                    ytiles.append(ytile)
                for ch in range(4):
                    pw = pw_pool.tile([128, NKT, 256], F32R, tag="pw")
                    nc.sync.dma_start(
                        pw[:],
                        wp_h[:, 256 * ch : 256 * (ch + 1)].rearrange(
                            "(kt p) n -> p kt n", p=128
                        ),
                    )
                    for tt in range(4):
                        n, sn = TT[tt], STT[tt]
                        acc = acc_pool.tile([128, TE], F32, tag="acc")
                        nc.tensor.matmul(
                            acc[0:sn, 0:256],
                            lhsT=ones[0:1, 0:sn],
                            rhs=bpe[0:1, 256 * ch : 256 * (ch + 1)],
                            start=True,
                            stop=False,
                        )
                        for kt in range(NKT):
                            nc.tensor.matmul(
                                acc[0:sn, 0:256],
                                lhsT=yh[:, kt, TOFF[tt] : TOFF[tt] + sn],
                                rhs=pw[:, kt, :],
                                start=False,
                                stop=(kt == NKT - 1),
                            )
                        nc.scalar.copy(
                            ytiles[tt][0:n, 256 * ch : 256 * (ch + 1)],
                            acc[0:n, 0:256],
                        )
                for tt in range(4):
                    n = TT[tt]
                    nc.sync.dma_start(
                        out_h[b, TOFF[tt] : TOFF[tt] + n, :], ytiles[tt][0:n, :]
                    )

    nc.compile()
    return nc


def _prep_inputs(x, W_attn, b_attn, W_proj, b_proj):
    wa = np.ascontiguousarray(W_attn, dtype=np.float32)
    wp = np.ascontiguousarray(W_proj, dtype=np.float32)
    bqs = (b_attn[:C].astype(np.float64) * 0.125).astype(np.float32)
    bk = np.ascontiguousarray(b_attn[C : 2 * C], dtype=np.float32)
    bv = b_attn[2 * C :].astype(np.float64)
    bpe = (b_proj.astype(np.float64) + bv @ W_proj.astype(np.float64)).astype(
        np.float32
    )
    # mask bias per key position: row 0 -> query in image1, row 1 -> image2
    mb = np.zeros((2, 512), dtype=np.float32)
    k = np.arange(T)
    img2 = (k >= T1 + 1).astype(np.float32)
    kzero = (k == 0).astype(np.float32)
    mb[0, :T] = kzero + img2          # q in img1: mask 1 at k=0 and k in img2
    mb[1, :T] = 1.0 - img2            # q in img2: mask 1 at k=0 and k in img1
    # device layout [p, kt, j]: mb_dev[p, kt, j] = mb[j, kt*128 + p]
    mb_dev = np.ascontiguousarray(mb.reshape(2, 4, 128).transpose(2, 1, 0))
    common = {"wa": wa, "wp": wp, "bqs": bqs, "bk": bk, "bpe": bpe, "mb": mb_dev}
    xs = np.ascontiguousarray(x, dtype=np.float32)
    in_maps = []
    for cidx in range(NCORES):
        m = dict(common)
        m["x"] = np.ascontiguousarray(xs[cidx * BL : (cidx + 1) * BL])
        in_maps.append(m)
    return in_maps


def _run(x, W_attn, b_attn, W_proj, b_proj, trace=False):
    if "nc" not in _cache:
        _cache["nc"] = _build()
    nc = _cache["nc"]
    in_maps = _prep_inputs(x, W_attn, b_attn, W_proj, b_proj)
    res = run_bass_kernel_spmd(
        nc, in_maps, core_ids=list(range(NCORES)), trace=trace
    )
    out = np.concatenate([r["out"] for r in res.results], axis=0)
    return out.astype(np.float32), res


def kernel(x, W_attn, b_attn, W_proj, b_proj):
    out, _ = _run(x, W_attn, b_attn, W_proj, b_proj, trace=False)
    return out


# revision 15
# speedup vs baseline: 2.8013x; 2.8013x over previous
"""CrossOnlyAttention Trainium2 kernel.

Data-parallel over batch: 64 batches -> 8 cores x 8 batches. Everything fp32r.

Per-core dataflow (per batch):
  x^T [C,T] in SBUF -> QKV projections:
     Q^T,K^T head-major [2 heads x 64, T] per head-pair (bias + 1/8 scale fused
     into the PSUM->SBUF copy), V token-major [T, 64] per head with a ones
     column appended (V_aug) so the PV matmul also produces the softmax
     denominator Z as row 64.
  Scores computed transposed S^T[k,q] = K^T.T @ Q^T (contraction=64, the two
     heads of a pair run on disjoint PE row groups). Softmax needs no
     max-subtraction (|logits| < ~4). The additive cross-mask reduces, up to a
     per-query constant that softmax cancels, to a per-KEY bias that only
     depends on which image the query is in -> folded into the exp() bias
     operand per free-dim slice ([1:235] vs [235:469]).
  PV: Y^T[65,q] = V_aug.T @ E accumulated over 4 k-tiles; row 64 = Z.
  Normalize: R=1/Z (DVE), broadcast R across 64 partitions with a K=1 matmul,
     DVE multiply -> Yhat^T [C,T] (odd heads shifted to partitions 64:128 via
     SBUF->SBUF DMA). V-bias is folded into b_proj on the host (attention rows
     sum to 1).
  Proj: out[t, c] accumulated over 8 cin tiles + a K=1 matmul adding
     b_proj_eff via a ones row.

fp32r ISA restrictions (walrus s3d3_mm_fp32r): every matmul operand free-dim
count and the PSUM dst free count must be EVEN, dst start_partition must be 0.
So token free dims are padded 469->470 (TE) and ragged stationary token slices
85->86 (STT); consumers only ever read the real 469/85 region, and the one
place padding could leak into real results (a fake key token entering the PV
reduction) is avoided because PV contracts over real partitions (85) only.
"""

import os
import sys

import numpy as np

for _p in (
    "/opt/trn_rl_repo",
    "/root/.axon_site",
    "/root/.axon_site/_ro/trn_rl_repo",
    "/root/.axon_site/_ro/pypackages",
):
    if os.path.isdir(_p) and _p not in sys.path:
        sys.path.append(_p)

import concourse.bass as bass  # noqa: E402,F401
import concourse.tile as tile  # noqa: E402
from concourse import bacc, mybir  # noqa: E402
from concourse.bass_utils import run_bass_kernel_spmd  # noqa: E402

B, T, C = 64, 469, 1024
H, HD = 16, 64
T1 = 234
NCORES = 8
BL = B // NCORES
F16 = mybir.dt.float16
F32 = mybir.dt.float32
TT = [128, 128, 128, 85]    # real token-tile sizes
STT = [128, 128, 128, 86]   # even-padded stationary slice sizes (fp32r ISA)
TOFF = [0, 128, 256, 384]
TE = 470                    # even-padded T for matmul free dims
NKT = 8  # cin contraction tiles (1024/128)
IDENT = mybir.ActivationFunctionType.Identity
EXP = mybir.ActivationFunctionType.Exp

_cache = {}


def _build():
    nc = bacc.Bacc(trn_type="TRN2", name="xattn")
    x_h = nc.dram_tensor("x", [BL, C, T], F16, kind="ExternalInput")
    wqk_h = nc.dram_tensor("wqk", [128, NKT, 2 * C], F16, kind="ExternalInput")
    wv_h = nc.dram_tensor("wv", [4, 128, NKT, 256], F16, kind="ExternalInput")
    wp_h = nc.dram_tensor("wp", [4, 128, NKT, 256], F16, kind="ExternalInput")
    bqs_h = nc.dram_tensor("bqs", [C], F32, kind="ExternalInput")
    bk_h = nc.dram_tensor("bk", [C], F32, kind="ExternalInput")
    bpe_h = nc.dram_tensor("bpe", [C], F16, kind="ExternalInput")
    mb_h = nc.dram_tensor("mb", [128, 4, 2], F32, kind="ExternalInput")
    out_h = nc.dram_tensor("out", [BL, T, C], F32, kind="ExternalOutput")

    with tile.TileContext(nc) as tc:
        with (
            tc.tile_pool(name="singles", bufs=1) as singles,
            tc.tile_pool(name="xy", bufs=2) as xy_pool,
            tc.tile_pool(name="qk", bufs=4) as qk_pool,
            tc.tile_pool(name="ep", bufs=3) as e_pool,
            tc.tile_pool(name="vw", bufs=2) as vw_pool,
            tc.tile_pool(name="pw", bufs=2) as pw_pool,
            tc.tile_pool(name="rp", bufs=2) as r_pool,
            tc.tile_pool(name="tp", bufs=2) as t_pool,
            tc.tile_pool(name="yp", bufs=4) as y_pool,
            tc.tile_pool(name="acc", bufs=3, space="PSUM") as acc_pool,
            tc.tile_pool(name="sp", bufs=3, space="PSUM") as s_pool,
            tc.tile_pool(name="pvp", bufs=2, space="PSUM") as pv_pool,
        ):
            # ---- resident constants ----
            wqk = singles.tile([128, NKT, 2 * C], F16)
            nc.sync.dma_start(wqk[:], wqk_h[:])
            ones32 = singles.tile([128, 128], F32)
            nc.vector.memset(ones32[:], 1.0)
            ones = singles.tile([128, 128], F16)
            nc.scalar.copy(ones[:], ones32[:])
            zeros32 = singles.tile([128, NKT], F32)
            nc.vector.memset(zeros32[:], 0.0)
            onz = singles.tile([128, H, 2], F32)
            nc.vector.memset(onz[:, :, 0:1], 1.0)
            nc.vector.memset(onz[:, :, 1:2], 0.0)
            vsb = singles.tile([128, 4, H, 66], F16)
            for _tt in range(4):
                nc.scalar.copy(vsb[:, _tt, :, 64:66], onz[:])
            bqs = singles.tile([128, NKT], F32)
            nc.sync.dma_start(bqs[:], bqs_h.ap().rearrange("(cb p) -> p cb", p=128))
            bk = singles.tile([128, NKT], F32)
            nc.sync.dma_start(bk[:], bk_h.ap().rearrange("(cb p) -> p cb", p=128))
            bpe = singles.tile([1, C], F16)
            nc.sync.dma_start(bpe[:], bpe_h.ap().unsqueeze(0))
            mb = singles.tile([128, 4, 2], F32)
            nc.sync.dma_start(mb[:], mb_h[:])

            for b in range(BL):
                xT = xy_pool.tile([128, NKT, TE], F16, tag="xy")
                for cs in range(NKT):
                    nc.sync.dma_start(
                        xT[:, cs, 0:T], x_h[b][128 * cs : 128 * (cs + 1), :]
                    )
                nc.scalar.copy(xT[:, :, T:TE], zeros32[:].unsqueeze(2))
                yh = xy_pool.tile([128, NKT, TE], F16, tag="xy")
                nc.scalar.copy(yh[:, :, T:TE], zeros32[:].unsqueeze(2))

                for c in range(4):
                    # V projection chunk: heads 4c..4c+3 (vcols 256c..256c+256)
                    vw = vw_pool.tile([128, NKT, 256], F16, tag="vw")
                    nc.sync.dma_start(vw[:], wv_h[c])
                    for tt in range(4):
                        n, sn = TT[tt], STT[tt]
                        acc = acc_pool.tile([128, TE], F32, tag="acc")
                        for kt in range(NKT):
                            nc.tensor.matmul(
                                acc[0:sn, 0:256],
                                lhsT=xT[:, kt, TOFF[tt] : TOFF[tt] + sn],
                                rhs=vw[:, kt, :],
                                start=(kt == 0),
                                stop=(kt == NKT - 1),
                            )
                        nc.scalar.copy(
                            vsb[0:n, tt, 4 * c : 4 * c + 4, 0:64],
                            acc[0:n, 0:256].rearrange("p (h d) -> p h d", h=4),
                        )

                    for hp in (2 * c, 2 * c + 1):
                        # Q^T / K^T for head pair hp (heads 2hp, 2hp+1)
                        qacc = acc_pool.tile([128, TE], F32, tag="acc")
                        for kt in range(NKT):
                            nc.tensor.matmul(
                                qacc[:, 0:TE],
                                lhsT=wqk[:, kt, 128 * hp : 128 * hp + 128],
                                rhs=xT[:, kt, 0:TE],
                                start=(kt == 0),
                                stop=(kt == NKT - 1),
                            )
                        qsb = qk_pool.tile([128, TE], F16, tag="qk")
                        nc.scalar.activation(
                            qsb[:], qacc[:, 0:TE], IDENT,
                            bias=bqs[:, hp : hp + 1], scale=0.125,
                        )
                        kacc = acc_pool.tile([128, TE], F32, tag="acc")
                        for kt in range(NKT):
                            nc.tensor.matmul(
                                kacc[:, 0:TE],
                                lhsT=wqk[:, kt, C + 128 * hp : C + 128 * hp + 128],
                                rhs=xT[:, kt, 0:TE],
                                start=(kt == 0),
                                stop=(kt == NKT - 1),
                            )
                        ksb = qk_pool.tile([128, TE], F16, tag="qk")
                        nc.scalar.activation(
                            ksb[:], kacc[:, 0:TE], IDENT,
                            bias=bk[:, hp : hp + 1], scale=1.0,
                        )

                        for sub in range(2):
                            h = 2 * hp + sub
                            r0 = 64 * sub
                            pv = pv_pool.tile([66, TE], F32, tag="pv")
                            for kt in range(4):
                                n, sn = TT[kt], STT[kt]
                                ko = TOFF[kt]
                                s = s_pool.tile([128, TE], F32, tag="s")
                                nc.tensor.matmul(
                                    s[0:sn, 0:TE],
                                    lhsT=ksb[r0 : r0 + 64, ko : ko + sn],
                                    rhs=qsb[r0 : r0 + 64, 0:TE],
                                    start=True,
                                    stop=True,
                                )
                                e = e_pool.tile([128, TE], F16, tag="e")
                                nc.scalar.activation(e[0:n, 0:1], s[0:n, 0:1], EXP)
                                nc.scalar.activation(
                                    e[0:n, 1 : T1 + 1], s[0:n, 1 : T1 + 1], EXP,
                                    bias=mb[0:n, kt, 0:1],
                                )
                                nc.scalar.activation(
                                    e[0:n, T1 + 1 : TE], s[0:n, T1 + 1 : TE], EXP,
                                    bias=mb[0:n, kt, 1:2],
                                )
                                nc.tensor.matmul(
                                    pv[:, 0:TE],
                                    lhsT=vsb[0:n, kt, h, 0:66],
                                    rhs=e[0:n, 0:TE],
                                    start=(kt == 0),
                                    stop=(kt == 3),
                                )
                            r = r_pool.tile([65, TE], F16, tag="r")
                            with nc.allow_low_precision(
                                reason="fp32r reciprocal feeds fp32r matmul"
                            ):
                                nc.vector.reciprocal(
                                    r[64:65, 0:TE], pv[64:65, 0:TE]
                                )
                            bc = s_pool.tile([128, TE], F32, tag="s")
                            nc.tensor.matmul(
                                bc[0:64, 0:TE],
                                lhsT=ones[64:65, 0:64],
                                rhs=r[64:65, 0:TE],
                                start=True,
                                stop=True,
                            )
                            bcs = t_pool.tile([64, T], F32, tag="bcs")
                            nc.scalar.copy(bcs[:, 0:T], bc[0:64, 0:T])
                            if sub == 0:
                                nc.vector.tensor_mul(
                                    yh[0:64, hp, 0:T], pv[0:64, 0:T], bcs[:, 0:T]
                                )
                            else:
                                tmp = t_pool.tile([64, T], F16, tag="tmp")
                                nc.vector.tensor_mul(
                                    tmp[:, 0:T], pv[0:64, 0:T], bcs[:, 0:T]
                                )
                                nc.sync.dma_start(yh[64:128, hp, 0:T], tmp[:, 0:T])

                # ---- output projection ----
                ytiles = []
                for _tt in range(4):
                    ytile = y_pool.tile([128, C], F32, tag="y", name=f"y{b}_{_tt}")
                    ytiles.append(ytile)
                for ch in range(4):
                    pw = pw_pool.tile([128, NKT, 256], F16, tag="pw")
                    nc.sync.dma_start(pw[:], wp_h[ch])
                    for tt in range(4):
                        n, sn = TT[tt], STT[tt]
                        acc = acc_pool.tile([128, TE], F32, tag="acc")
                        nc.tensor.matmul(
                            acc[0:sn, 0:256],
                            lhsT=ones[0:1, 0:sn],
                            rhs=bpe[0:1, 256 * ch : 256 * (ch + 1)],
                            start=True,
                            stop=False,
                        )
                        for kt in range(NKT):
                            nc.tensor.matmul(
                                acc[0:sn, 0:256],
                                lhsT=yh[:, kt, TOFF[tt] : TOFF[tt] + sn],
                                rhs=pw[:, kt, :],
                                start=False,
                                stop=(kt == NKT - 1),
                            )
                        nc.scalar.copy(
                            ytiles[tt][0:n, 256 * ch : 256 * (ch + 1)],
                            acc[0:n, 0:256],
                        )
                for tt in range(4):
                    n = TT[tt]
                    nc.sync.dma_start(
                        out_h[b, TOFF[tt] : TOFF[tt] + n, :], ytiles[tt][0:n, :]
                    )

    nc.compile()
    return nc


def _pack_w(w):
    # [C, n] -> [128, NKT, n] with w_packed[p, kt, j] = w[kt*128 + p, j]
    n = w.shape[1]
    return np.ascontiguousarray(
        w.reshape(NKT, 128, n).transpose(1, 0, 2), dtype=np.float16
    )


def _prep_inputs(x, W_attn, b_attn, W_proj, b_proj):
    wqk = _pack_w(np.asarray(W_attn[:, : 2 * C]))
    wv = np.stack(
        [
            _pack_w(np.asarray(W_attn[:, 2 * C + 256 * c : 2 * C + 256 * (c + 1)]))
            for c in range(4)
        ]
    )
    wpk = np.stack(
        [
            _pack_w(np.asarray(W_proj[:, 256 * c : 256 * (c + 1)]))
            for c in range(4)
        ]
    )
    bqs = (b_attn[:C].astype(np.float64) * 0.125).astype(np.float32)
    bk = np.ascontiguousarray(b_attn[C : 2 * C], dtype=np.float32)
    bv = b_attn[2 * C :].astype(np.float64)
    bpe = (b_proj.astype(np.float64) + bv @ W_proj.astype(np.float64)).astype(
        np.float16
    )
    # mask bias per key position: row 0 -> query in image1, row 1 -> image2
    mb = np.zeros((2, 512), dtype=np.float32)
    k = np.arange(T)
    img2 = (k >= T1 + 1).astype(np.float32)
    kzero = (k == 0).astype(np.float32)
    mb[0, :T] = kzero + img2          # q in img1: mask 1 at k=0 and k in img2
    mb[1, :T] = 1.0 - img2            # q in img2: mask 1 at k=0 and k in img1
    # device layout [p, kt, j]: mb_dev[p, kt, j] = mb[j, kt*128 + p]
    mb_dev = np.ascontiguousarray(mb.reshape(2, 4, 128).transpose(2, 1, 0))
    common = {
        "wqk": wqk, "wv": wv, "wp": wpk, "bqs": bqs, "bk": bk,
        "bpe": bpe, "mb": mb_dev,
    }
    # x -> [B, C, T] fp16 (pre-transposed so the device DMA is near-linear)
    xs = np.ascontiguousarray(
        np.asarray(x).astype(np.float16).transpose(0, 2, 1)
    )
    in_maps = []
    for cidx in range(NCORES):
        m = dict(common)
        m["x"] = np.ascontiguousarray(xs[cidx * BL : (cidx + 1) * BL])
        in_maps.append(m)
    return in_maps


def _run(x, W_attn, b_attn, W_proj, b_proj, trace=False):
    if "nc" not in _cache:
        _cache["nc"] = _build()
    nc = _cache["nc"]
    in_maps = _prep_inputs(x, W_attn, b_attn, W_proj, b_proj)
    res = run_bass_kernel_spmd(
        nc, in_maps, core_ids=list(range(NCORES)), trace=trace
    )
    out = np.concatenate([r["out"] for r in res.results], axis=0)
    return out.astype(np.float32), res


def kernel(x, W_attn, b_attn, W_proj, b_proj):
    out, _ = _run(x, W_attn, b_attn, W_proj, b_proj, trace=False)
    return out


# revision 17
# speedup vs baseline: 4.2848x; 1.5296x over previous
"""CrossOnlyAttention Trainium2 kernel.

Data-parallel over batch: 64 batches -> 8 cores x 8 batches. Everything fp32r.

Per-core dataflow (per batch):
  x^T [C,T] in SBUF -> QKV projections:
     Q^T,K^T head-major [2 heads x 64, T] per head-pair (bias + 1/8 scale fused
     into the PSUM->SBUF copy), V token-major [T, 64] per head with a ones
     column appended (V_aug) so the PV matmul also produces the softmax
     denominator Z as row 64.
  Scores computed transposed S^T[k,q] = K^T.T @ Q^T (contraction=64, the two
     heads of a pair run on disjoint PE row groups). Softmax needs no
     max-subtraction (|logits| < ~4). The additive cross-mask reduces, up to a
     per-query constant that softmax cancels, to a per-KEY bias that only
     depends on which image the query is in -> folded into the exp() bias
     operand per free-dim slice ([1:235] vs [235:469]).
  PV: Y^T[65,q] = V_aug.T @ E accumulated over 4 k-tiles; row 64 = Z.
  Normalize: R=1/Z (DVE), broadcast R across 64 partitions with a K=1 matmul,
     DVE multiply -> Yhat^T [C,T] (odd heads shifted to partitions 64:128 via
     SBUF->SBUF DMA). V-bias is folded into b_proj on the host (attention rows
     sum to 1).
  Proj: out[t, c] accumulated over 8 cin tiles + a K=1 matmul adding
     b_proj_eff via a ones row.

fp32r ISA restrictions (walrus s3d3_mm_fp32r): every matmul operand free-dim
count and the PSUM dst free count must be EVEN, dst start_partition must be 0.
So token free dims are padded 469->470 (TE) and ragged stationary token slices
85->86 (STT); consumers only ever read the real 469/85 region, and the one
place padding could leak into real results (a fake key token entering the PV
reduction) is avoided because PV contracts over real partitions (85) only.
"""

import os
import sys

import numpy as np

for _p in (
    "/opt/trn_rl_repo",
    "/root/.axon_site",
    "/root/.axon_site/_ro/trn_rl_repo",
    "/root/.axon_site/_ro/pypackages",
):
    if os.path.isdir(_p) and _p not in sys.path:
        sys.path.append(_p)

import concourse.bass as bass  # noqa: E402,F401
import concourse.tile as tile  # noqa: E402
from concourse import bacc, mybir  # noqa: E402
from concourse.bass_utils import run_bass_kernel_spmd  # noqa: E402

B, T, C = 64, 469, 1024
H, HD = 16, 64
T1 = 234
NCORES = 8
BL = B // NCORES
F16 = mybir.dt.float16
F32 = mybir.dt.float32
TT = [128, 128, 128, 85]    # real token-tile sizes
STT = [128, 128, 128, 86]   # even-padded stationary slice sizes (fp32r ISA)
TOFF = [0, 128, 256, 384]
TE = 470                    # even-padded T for matmul free dims
NKT = 8  # cin contraction tiles (1024/128)
IDENT = mybir.ActivationFunctionType.Identity
EXP = mybir.ActivationFunctionType.Exp

_cache = {}


def _build():
    nc = bacc.Bacc(trn_type="TRN2", name="xattn")
    x_h = nc.dram_tensor("x", [BL, C, T], F16, kind="ExternalInput")
    wqk_h = nc.dram_tensor("wqk", [128, NKT, 2 * C], F16, kind="ExternalInput")
    wv_h = nc.dram_tensor("wv", [4, 128, NKT, 256], F16, kind="ExternalInput")
    wp_h = nc.dram_tensor("wp", [4, 128, NKT, 256], F16, kind="ExternalInput")
    bqs_h = nc.dram_tensor("bqs", [C], F32, kind="ExternalInput")
    bk_h = nc.dram_tensor("bk", [C], F32, kind="ExternalInput")
    bpe_h = nc.dram_tensor("bpe", [C], F16, kind="ExternalInput")
    mb_h = nc.dram_tensor("mb", [128, 4, 2], F32, kind="ExternalInput")
    out_h = nc.dram_tensor("out", [BL, T, C], F32, kind="ExternalOutput")

    with tile.TileContext(nc) as tc:
        with (
            tc.tile_pool(name="singles", bufs=1) as singles,
            tc.tile_pool(name="xy", bufs=2) as xy_pool,
            tc.tile_pool(name="qk", bufs=4) as qk_pool,
            tc.tile_pool(name="ep", bufs=3) as e_pool,
            tc.tile_pool(name="vw", bufs=2) as vw_pool,
            tc.tile_pool(name="pw", bufs=2) as pw_pool,
            tc.tile_pool(name="rp", bufs=2) as r_pool,
            tc.tile_pool(name="tp", bufs=2) as t_pool,
            tc.tile_pool(name="yp", bufs=4) as y_pool,
            tc.tile_pool(name="acc", bufs=3, space="PSUM") as acc_pool,
            tc.tile_pool(name="sp", bufs=3, space="PSUM") as s_pool,
            tc.tile_pool(name="pvp", bufs=2, space="PSUM") as pv_pool,
        ):
            # ---- resident constants ----
            wqk = singles.tile([128, NKT, 2 * C], F16)
            nc.sync.dma_start(wqk[:], wqk_h[:])
            ones32 = singles.tile([128, 128], F32)
            nc.vector.memset(ones32[:], 1.0)
            ones = singles.tile([128, 128], F16)
            nc.scalar.copy(ones[:], ones32[:])
            zeros32 = singles.tile([128, NKT], F32)
            nc.vector.memset(zeros32[:], 0.0)
            onz = singles.tile([128, H, 2], F32)
            nc.vector.memset(onz[:, :, 0:1], 1.0)
            nc.vector.memset(onz[:, :, 1:2], 0.0)
            vsb = singles.tile([128, 4, H, 66], F16)
            for _tt in range(4):
                nc.scalar.copy(vsb[:, _tt, :, 64:66], onz[:])
            bqs = singles.tile([128, NKT], F32)
            nc.sync.dma_start(bqs[:], bqs_h.ap().rearrange("(cb p) -> p cb", p=128))
            bk = singles.tile([128, NKT], F32)
            nc.sync.dma_start(bk[:], bk_h.ap().rearrange("(cb p) -> p cb", p=128))
            bpe = singles.tile([1, C], F16)
            nc.sync.dma_start(bpe[:], bpe_h.ap().unsqueeze(0))
            mb = singles.tile([128, 4, 2], F32)
            nc.sync.dma_start(mb[:], mb_h[:])

            for b in range(BL):
                xT = xy_pool.tile([128, NKT, TE], F16, tag="xy")
                for cs in range(NKT):
                    nc.sync.dma_start(
                        xT[:, cs, 0:T], x_h[b][128 * cs : 128 * (cs + 1), :]
                    )
                nc.scalar.copy(xT[:, :, T:TE], zeros32[:].unsqueeze(2))
                yh = xy_pool.tile([128, NKT, TE], F16, tag="xy")
                nc.scalar.copy(yh[:, :, T:TE], zeros32[:].unsqueeze(2))

                pend = None

                def norm_tail(pv, r, hp, sub, yh=yh):
                    bc = s_pool.tile([128, TE], F32, tag="s")
                    nc.tensor.matmul(
                        bc[0:64, 0:TE],
                        lhsT=ones[64:65, 0:64],
                        rhs=r[64:65, 0:TE],
                        start=True,
                        stop=True,
                    )
                    bcs = t_pool.tile([64, T], F32, tag="bcs")
                    nc.vector.tensor_copy(bcs[:, 0:T], bc[0:64, 0:T])
                    if sub == 0:
                        nc.vector.tensor_mul(
                            yh[0:64, hp, 0:T], pv[0:64, 0:T], bcs[:, 0:T]
                        )
                    else:
                        tmp = t_pool.tile([64, T], F16, tag="tmp")
                        nc.vector.tensor_mul(
                            tmp[:, 0:T], pv[0:64, 0:T], bcs[:, 0:T]
                        )
                        nc.sync.dma_start(yh[64:128, hp, 0:T], tmp[:, 0:T])

                for c in range(4):
                    # V projection chunk: heads 4c..4c+3 (vcols 256c..256c+256)
                    vw = vw_pool.tile([128, NKT, 256], F16, tag="vw")
                    nc.sync.dma_start(vw[:], wv_h[c])
                    for tt in range(4):
                        n, sn = TT[tt], STT[tt]
                        acc = acc_pool.tile([128, TE], F32, tag="acc")
                        for kt in range(NKT):
                            nc.tensor.matmul(
                                acc[0:sn, 0:256],
                                lhsT=xT[:, kt, TOFF[tt] : TOFF[tt] + sn],
                                rhs=vw[:, kt, :],
                                start=(kt == 0),
                                stop=(kt == NKT - 1),
                            )
                        nc.vector.tensor_copy(
                            vsb[0:n, tt, 4 * c : 4 * c + 4, 0:64],
                            acc[0:n, 0:256].rearrange("p (h d) -> p h d", h=4),
                        )

                    for hp in (2 * c, 2 * c + 1):
                        # Q^T / K^T for head pair hp (heads 2hp, 2hp+1)
                        qacc = acc_pool.tile([128, TE], F32, tag="acc")
                        for kt in range(NKT):
                            nc.tensor.matmul(
                                qacc[:, 0:TE],
                                lhsT=wqk[:, kt, 128 * hp : 128 * hp + 128],
                                rhs=xT[:, kt, 0:TE],
                                start=(kt == 0),
                                stop=(kt == NKT - 1),
                            )
                        qsb = qk_pool.tile([128, TE], F16, tag="qk")
                        nc.scalar.activation(
                            qsb[:], qacc[:, 0:TE], IDENT,
                            bias=bqs[:, hp : hp + 1], scale=0.125,
                        )
                        kacc = acc_pool.tile([128, TE], F32, tag="acc")
                        for kt in range(NKT):
                            nc.tensor.matmul(
                                kacc[:, 0:TE],
                                lhsT=wqk[:, kt, C + 128 * hp : C + 128 * hp + 128],
                                rhs=xT[:, kt, 0:TE],
                                start=(kt == 0),
                                stop=(kt == NKT - 1),
                            )
                        ksb = qk_pool.tile([128, TE], F16, tag="qk")
                        nc.scalar.activation(
                            ksb[:], kacc[:, 0:TE], IDENT,
                            bias=bk[:, hp : hp + 1], scale=1.0,
                        )

                        for sub in range(2):
                            h = 2 * hp + sub
                            r0 = 64 * sub
                            pv = pv_pool.tile([66, TE], F32, tag="pv")
                            for kt in range(4):
                                n, sn = TT[kt], STT[kt]
                                ko = TOFF[kt]
                                s = s_pool.tile([128, TE], F32, tag="s")
                                nc.tensor.matmul(
                                    s[0:sn, 0:TE],
                                    lhsT=ksb[r0 : r0 + 64, ko : ko + sn],
                                    rhs=qsb[r0 : r0 + 64, 0:TE],
                                    start=True,
                                    stop=True,
                                )
                                e = e_pool.tile([128, TE], F16, tag="e")
                                nc.scalar.activation(e[0:n, 0:1], s[0:n, 0:1], EXP)
                                nc.scalar.activation(
                                    e[0:n, 1 : T1 + 1], s[0:n, 1 : T1 + 1], EXP,
                                    bias=mb[0:n, kt, 0:1],
                                )
                                nc.scalar.activation(
                                    e[0:n, T1 + 1 : TE], s[0:n, T1 + 1 : TE], EXP,
                                    bias=mb[0:n, kt, 1:2],
                                )
                                nc.tensor.matmul(
                                    pv[:, 0:TE],
                                    lhsT=vsb[0:n, kt, h, 0:66],
                                    rhs=e[0:n, 0:TE],
                                    start=(kt == 0),
                                    stop=(kt == 3),
                                )
                            # start the reciprocal now (DVE), but defer the
                            # PE-side broadcast by one head so the in-order PE
                            # queue never waits on it
                            r = r_pool.tile([65, TE], F16, tag="r")
                            with nc.allow_low_precision(
                                reason="softmax denominators fit fp16"
                            ):
                                nc.vector.reciprocal(
                                    r[64:65, 0:TE], pv[64:65, 0:TE]
                                )
                            if pend is not None:
                                norm_tail(*pend)
                            pend = (pv, r, hp, sub)

                if pend is not None:
                    norm_tail(*pend)
                    pend = None

                # ---- output projection ----
                ytiles = []
                for _tt in range(4):
                    ytile = y_pool.tile([128, C], F32, tag="y", name=f"y{b}_{_tt}")
                    ytiles.append(ytile)
                for ch in range(4):
                    pw = pw_pool.tile([128, NKT, 256], F16, tag="pw")
                    nc.sync.dma_start(pw[:], wp_h[ch])
                    for tt in range(4):
                        n, sn = TT[tt], STT[tt]
                        acc = acc_pool.tile([128, TE], F32, tag="acc")
                        nc.tensor.matmul(
                            acc[0:sn, 0:256],
                            lhsT=ones[0:1, 0:sn],
                            rhs=bpe[0:1, 256 * ch : 256 * (ch + 1)],
                            start=True,
                            stop=False,
                        )
                        for kt in range(NKT):
                            nc.tensor.matmul(
                                acc[0:sn, 0:256],
                                lhsT=yh[:, kt, TOFF[tt] : TOFF[tt] + sn],
                                rhs=pw[:, kt, :],
                                start=False,
                                stop=(kt == NKT - 1),
                            )
                        nc.vector.tensor_copy(
                            ytiles[tt][0:n, 256 * ch : 256 * (ch + 1)],
                            acc[0:n, 0:256],
                        )
                for tt in range(4):
                    n = TT[tt]
                    nc.sync.dma_start(
                        out_h[b, TOFF[tt] : TOFF[tt] + n, :], ytiles[tt][0:n, :]
                    )

    nc.compile()
    return nc


def _pack_w(w):
    # [C, n] -> [128, NKT, n] with w_packed[p, kt, j] = w[kt*128 + p, j]
    n = w.shape[1]
    return np.ascontiguousarray(
        w.reshape(NKT, 128, n).transpose(1, 0, 2), dtype=np.float16
    )


def _prep_inputs(x, W_attn, b_attn, W_proj, b_proj):
    wqk = _pack_w(np.asarray(W_attn[:, : 2 * C]))
    wv = np.stack(
        [
            _pack_w(np.asarray(W_attn[:, 2 * C + 256 * c : 2 * C + 256 * (c + 1)]))
            for c in range(4)
        ]
    )
    wpk = np.stack(
        [
            _pack_w(np.asarray(W_proj[:, 256 * c : 256 * (c + 1)]))
            for c in range(4)
        ]
    )
    bqs = (b_attn[:C].astype(np.float64) * 0.125).astype(np.float32)
    bk = np.ascontiguousarray(b_attn[C : 2 * C], dtype=np.float32)
    bv = b_attn[2 * C :].astype(np.float64)
    bpe = (b_proj.astype(np.float64) + bv @ W_proj.astype(np.float64)).astype(
        np.float16
    )
    # mask bias per key position: row 0 -> query in image1, row 1 -> image2
    mb = np.zeros((2, 512), dtype=np.float32)
    k = np.arange(T)
    img2 = (k >= T1 + 1).astype(np.float32)
    kzero = (k == 0).astype(np.float32)
    mb[0, :T] = kzero + img2          # q in img1: mask 1 at k=0 and k in img2
    mb[1, :T] = 1.0 - img2            # q in img2: mask 1 at k=0 and k in img1
    # device layout [p, kt, j]: mb_dev[p, kt, j] = mb[j, kt*128 + p]
    mb_dev = np.ascontiguousarray(mb.reshape(2, 4, 128).transpose(2, 1, 0))
    common = {
        "wqk": wqk, "wv": wv, "wp": wpk, "bqs": bqs, "bk": bk,
        "bpe": bpe, "mb": mb_dev,
    }
    # x -> [B, C, T] fp16 (pre-transposed so the device DMA is near-linear)
    xs = np.ascontiguousarray(
        np.asarray(x).astype(np.float16).transpose(0, 2, 1)
    )
    in_maps = []
    for cidx in range(NCORES):
        m = dict(common)
        m["x"] = np.ascontiguousarray(xs[cidx * BL : (cidx + 1) * BL])
        in_maps.append(m)
    return in_maps


def _run(x, W_attn, b_attn, W_proj, b_proj, trace=False):
    if "nc" not in _cache:
        _cache["nc"] = _build()
    nc = _cache["nc"]
    in_maps = _prep_inputs(x, W_attn, b_attn, W_proj, b_proj)
    res = run_bass_kernel_spmd(
        nc, in_maps, core_ids=list(range(NCORES)), trace=trace
    )
    out = np.concatenate([r["out"] for r in res.results], axis=0)
    return out.astype(np.float32), res


def kernel(x, W_attn, b_attn, W_proj, b_proj):
    out, _ = _run(x, W_attn, b_attn, W_proj, b_proj, trace=False)
    return out
